# revision 1
# baseline (speedup 1.0000x reference)
"""Multi-head attention + residual + LayerNorm on 8 Trainium2 NeuronCores.

Sharding: core c in 0..7 handles batch b = c//4 and query-row quarter
r = c%4 (rows 512r..512r+512 of S=2048), with ALL 16 heads.  key/value
are replicated to every core (host-side staging); each core computes the
full-sequence K^T and V projections itself — measured collectives on this
stack cost ~130us per 2MB AllGather, far more than the ~70us of redundant
PE work, and the local pipeline keeps the PE clock warm.

Per core:
  - transpose x tiles on PE (fp32r, via identity), project:
      Q^T [1024, 512] (own rows),  K^T [1024, 2048] -> local DRAM,
      V [2048, 8, 130] pair-blocks with ones columns -> local DRAM
  - per head pair p, per sk chunk c: S^T = K_h Q_h^T  (PSUM) -> exp
    (ACT, scale 1/8) -> U^T accumulation with lhsT = V_aug; the ones
    column makes row 64 the softmax denominators
  - normalize: reciprocal of sums row, PE K=1 broadcast to 64
    partitions, multiply -> ctx^T [128, 8, 512]
  - out = ctx @ Wo + bo + residual -> LayerNorm -> y rows [512, 1024]

All matmuls in float32r (full-rate fp32 PE path, ~2e-4 rel err).
"""

import sys

if "/opt/trn_rl_repo" not in sys.path:
    sys.path.insert(0, "/opt/trn_rl_repo")

import numpy as np

import concourse.bacc as bacc
import concourse.bass as bass
import concourse.mybir as mybir
import concourse.tile as tile
from concourse.bass import ds, ts
from concourse.bass_utils import run_bass_kernel_spmd

FP32R = mybir.dt.float32r
FP32 = mybir.dt.float32
AF = mybir.ActivationFunctionType
ALU = mybir.AluOpType

N_CORES = 8
B = 2
S = 2048
D = 1024
H = 16
DK = 64
P = 128

SL = S // 4  # 512 local query rows per core
KC = D // P  # 8 contraction chunks over d_model
SQ = SL // P  # 4 sq subchunks of 128 (per 512-row block)
CH = S // P  # 16 sk chunks
PAIRS = H // 2  # 8 head pairs
NB = 4  # 512-row blocks of the full sequence
EPS = 1e-5

_NC_CACHE = {}


def build_nc():
    nc = bacc.Bacc(num_devices=N_CORES)

    xq_d = nc.dram_tensor("xq", [SL, D], FP32R, kind="ExternalInput")
    xk_d = nc.dram_tensor("xk", [S, D], FP32R, kind="ExternalInput")
    xv_d = nc.dram_tensor("xv", [S, D], FP32R, kind="ExternalInput")
    wq_d = nc.dram_tensor("wq", [D, D], FP32R, kind="ExternalInput")
    wk_d = nc.dram_tensor("wk", [D, D], FP32R, kind="ExternalInput")
    wv_d = nc.dram_tensor("wv", [D, D], FP32R, kind="ExternalInput")
    wo_d = nc.dram_tensor("wo", [D, D], FP32R, kind="ExternalInput")
    bq_d = nc.dram_tensor("bq", [D], FP32, kind="ExternalInput")
    bk_d = nc.dram_tensor("bk", [D], FP32, kind="ExternalInput")
    bv_d = nc.dram_tensor("bv", [D], FP32, kind="ExternalInput")
    bo_d = nc.dram_tensor("bo", [D], FP32, kind="ExternalInput")
    gam_d = nc.dram_tensor("gam", [D], FP32, kind="ExternalInput")
    bet_d = nc.dram_tensor("bet", [D], FP32, kind="ExternalInput")
    ident_d = nc.dram_tensor("ident", [P, P], FP32R, kind="ExternalInput")
    ones_d = nc.dram_tensor("ones", [P, 64], FP32R, kind="ExternalInput")

    y_d = nc.dram_tensor("y", [SL, D], FP32, kind="ExternalOutput")

    # local DRAM for the full-sequence K^T and augmented V
    kt_d = nc.dram_tensor("ktf", [D, S], FP32R)
    vf_d = nc.dram_tensor("vf", [S, PAIRS, 130], FP32R)

    with tile.TileContext(nc) as tc:
        with (
            tc.tile_pool(name="consts", bufs=1) as consts,
            tc.tile_pool(name="big", bufs=1) as big,
            tc.tile_pool(name="xtp", bufs=1) as xtp,
            tc.tile_pool(name="wide", bufs=1) as wide,
            tc.tile_pool(name="xnp", bufs=4) as xnp,
            tc.tile_pool(name="wpool", bufs=1) as wpool,
            tc.tile_pool(name="stream", bufs=3) as stream,
            tc.tile_pool(name="kttp", bufs=7) as kttp,
            tc.tile_pool(name="etp", bufs=4) as etp,
            tc.tile_pool(name="vat", bufs=2) as vatp,
            tc.tile_pool(name="small", bufs=2) as small,
            tc.tile_pool(name="psA", bufs=3, space="PSUM") as psA,
            tc.tile_pool(name="psAcc", bufs=2, space="PSUM") as psAcc,
            tc.tile_pool(name="psB", bufs=1, space="PSUM") as psB,
        ):
            # ---- constants ----
            ident = consts.tile([P, P], FP32R)
            nc.sync.dma_start(ident[:], ident_d[:])
            ones64 = consts.tile([P, 64], FP32R)
            nc.sync.dma_start(ones64[:], ones_d[:])
            bq_sb = consts.tile([P, KC], FP32)
            nc.sync.dma_start(bq_sb[:], bq_d.rearrange("(m q) -> q m", q=P))
            bk_sb = consts.tile([P, KC], FP32)
            nc.sync.dma_start(bk_sb[:], bk_d.rearrange("(m q) -> q m", q=P))

            def bcast_load(src, tag):
                t = consts.tile([P, D], FP32, tag=tag)
                ap = bass.AP(tensor=src, offset=0, ap=[[0, P], [1, D]])
                nc.gpsimd.dma_start(out=t[:], in_=ap)
                return t

            bv_b = bcast_load(bv_d, "bv_b")
            bo_b = bcast_load(bo_d, "bo_b")
            gam_b = bcast_load(gam_d, "gam_b")
            bet_b = bcast_load(bet_d, "bet_b")
            eps_t = consts.tile([P, 1], FP32)
            nc.vector.memset(eps_t[:], EPS)

            def load_xT(x_d, row0):
                """x rows [row0:row0+512] -> x^T SBUF [128, KC, 512]."""
                xT = xtp.tile([P, KC, SL], FP32R, tag="xT")
                for i in range(SQ):
                    xt = xnp.tile([P, D], FP32R, tag="xnat")
                    nc.sync.dma_start(xt[:], x_d[ds(row0 + i * P, P), :])
                    for j in range(KC):
                        pt = psA.tile([P, P], FP32R, tag="mm")
                        nc.tensor.transpose(pt[:], xt[:, ts(j, P)], ident[:])
                        nc.vector.tensor_copy(xT[:, j, ts(i, P)], pt[:])
                return xT

            # ---- K^T full sequence -> kt_d, block by block ----
            kt_dr = kt_d.rearrange("(m q) s -> q m s", q=P)
            wk_sb = wpool.tile([P, KC, D], FP32R, tag="wrhs")
            for k in range(KC):
                nc.sync.dma_start(wk_sb[:, k, :], wk_d[ts(k, P), :])
            for blk in range(NB):
                xkT = load_xT(xk_d, blk * SL)
                for m in range(KC):
                    pp = psA.tile([P, SL], FP32, tag="mm")
                    for k in range(KC):
                        nc.tensor.matmul(
                            pp[:],
                            wk_sb[:, k, ts(m, P)],
                            xkT[:, k, :],
                            start=(k == 0),
                            stop=(k == KC - 1),
                        )
                    kev = stream.tile([P, SL], FP32R, tag="kev")
                    nc.scalar.activation(
                        out=kev[:],
                        in_=pp[:],
                        func=AF.Identity,
                        bias=bk_sb[:, m : m + 1],
                    )
                    nc.sync.dma_start(kt_dr[:, m, ds(blk * SL, SL)], kev[:])

            # ---- V full sequence -> vf_d (pair-augmented layout) ----
            vf_dr = vf_d.rearrange("(i q) p e -> q i p e", q=P)
            wv_sb = wpool.tile([P, KC, D], FP32R, tag="wrhs")
            for k in range(KC):
                nc.sync.dma_start(wv_sb[:, k, :], wv_d[ts(k, P), :])
            for blk in range(NB):
                xvT = load_xT(xv_d, blk * SL)
                for n in range(2):
                    for i in range(SQ):
                        pp = psA.tile([P, 512], FP32, tag="mm")
                        for k in range(KC):
                            nc.tensor.matmul(
                                pp[:],
                                xvT[:, k, ts(i, P)],
                                wv_sb[:, k, ds(n * 512, 512)],
                                start=(k == 0),
                                stop=(k == KC - 1),
                            )
                        # vtmp holds [4 pairs x (V_even |1| V_odd |1)] = 520 cols
                        vtmp = stream.tile([P, 4, 130], FP32R, tag="vtmp")
                        vdst = vtmp[:].rearrange("q pl (j e) -> q pl j e", e=65)
                        nc.vector.tensor_tensor(
                            vdst[:, :, :, 0:64],
                            pp[:].rearrange("q (pl j e) -> q pl j e", pl=4, j=2),
                            bv_b[:, ds(n * 512, 512)].rearrange(
                                "q (pl j e) -> q pl j e", pl=4, j=2
                            ),
                            ALU.add,
                        )
                        nc.vector.tensor_copy(
                            vdst[:, :, :, 64:65], ones64[:, 0:8, None].rearrange(
                                "q (pl j) o -> q pl j o", pl=4
                            )
                        )
                        ii = blk * SQ + i
                        nc.sync.dma_start(vf_dr[:, ii, ds(n * 4, 4), :], vtmp[:])

            # ---- Q^T (own rows) ----
            xqT = load_xT(xq_d, 0)
            qt_sb = big.tile([P, KC, SL], FP32R, tag="qt")
            wq_sb = wpool.tile([P, KC, D], FP32R, tag="wrhs")
            for k in range(KC):
                nc.sync.dma_start(wq_sb[:, k, :], wq_d[ts(k, P), :])
            for m in range(KC):
                pp = psA.tile([P, SL], FP32, tag="mm")
                for k in range(KC):
                    nc.tensor.matmul(
                        pp[:],
                        wq_sb[:, k, ts(m, P)],
                        xqT[:, k, :],
                        start=(k == 0),
                        stop=(k == KC - 1),
                    )
                nc.scalar.activation(
                    out=qt_sb[:, m, :],
                    in_=pp[:],
                    func=AF.Identity,
                    bias=bq_sb[:, m : m + 1],
                )

            # ---- attention ----
            ctx_sb = big.tile([P, PAIRS, SL], FP32R, tag="ctx")
            vf_blk = vf_d.rearrange("(b i q) p e -> q b i p e", b=NB, q=P)

            def emit_normalize(np_, uA, uB):
                # rows 0..63 of ut / row 64 -> ctx_sb[:, np_, :]
                for j, ut in enumerate((uA, uB)):
                    rec = small.tile([P, SL], FP32R, tag="rec")
                    with nc.allow_low_precision(
                        reason="float32r is bit-identical to float32 in SBUF"
                    ):
                        nc.vector.reciprocal(out=rec[64:65, :], in_=ut[64:65, :])
                    bc = psB.tile([P, SL], FP32, tag="bc")
                    nc.tensor.matmul(
                        bc[0:64, :],
                        ones64[64:65, :],
                        rec[64:65, :],
                        start=True,
                        stop=True,
                    )
                    bc_sb = small.tile([P, SL], FP32, tag="bcs")
                    nc.vector.tensor_copy(bc_sb[0:64, :], bc[0:64, :])
                    if j == 0:
                        nc.vector.tensor_tensor(
                            ctx_sb[0:64, np_, :], ut[0:64, :], bc_sb[0:64, :], ALU.mult
                        )
                    else:
                        ctmp = small.tile([P, SL], FP32R, tag="ctmp")
                        nc.vector.tensor_tensor(
                            ctmp[0:64, :], ut[0:64, :], bc_sb[0:64, :], ALU.mult
                        )
                        # partition shift 0-63 -> 64-127 via SBUF-SBUF DMA
                        nc.sync.dma_start(ctx_sb[64:128, np_, :], ctmp[0:64, :])

            def prefetch_pair(pp_):
                vt = vatp.tile([P, NB, SQ, 130], FP32R, tag="vat", name=f"vt_{pp_}")
                for b in range(NB):
                    nc.sync.dma_start(vt[:, b], vf_blk[:, b, :, pp_, :])
                ktts = []
                for sb4 in range(NB):
                    kq = kttp.tile(
                        [P, SL], FP32R, tag="ktt", name=f"ktt_{pp_}_{sb4}"
                    )
                    nc.sync.dma_start(kq[:], kt_dr[:, pp_, ds(sb4 * SL, SL)])
                    ktts.append(kq)
                return vt, ktts

            tiles = {0: prefetch_pair(0)}
            norm_pend = None
            for p in range(PAIRS):
                utA = psAcc.tile([P, SL], FP32, tag="accA")
                utB = psAcc.tile([P, SL], FP32, tag="accB")
                vt, ktts = tiles.pop(p)
                # software pipeline: issue S^T/exp for chunk c+1 before the
                # U^T matmuls of chunk c, so the in-order PE never stalls on
                # ACT; the previous pair's normalize is likewise deferred into
                # this pair's stream so its PE broadcast never waits on DVE.
                pend = None
                for c in range(CH):
                    ktt = ktts[c // SQ][:, ts(c % SQ, P)]
                    ets = []
                    for j in range(2):
                        st = psA.tile([P, SL], FP32, tag="mm")
                        nc.tensor.matmul(
                            st[:],
                            ktt[ds(j * 64, 64), :],
                            qt_sb[ds(j * 64, 64), p, :],
                            start=True,
                            stop=True,
                        )
                        et = etp.tile([P, SL], FP32R, tag="et")
                        nc.scalar.activation(
                            out=et[:], in_=st[:], func=AF.Exp, scale=0.125
                        )
                        ets.append(et)
                    if c == 7 and norm_pend is not None:
                        emit_normalize(*norm_pend)
                        norm_pend = None
                    if c == 4 and p + 1 < PAIRS:
                        tiles[p + 1] = prefetch_pair(p + 1)
                    if pend is not None:
                        pc, pets, pv = pend
                        for j, ut in enumerate((utA, utB)):
                            nc.tensor.matmul(
                                ut[:65, :],
                                pv[:, ds(j * 65, 65)],
                                pets[j][:],
                                start=(pc == 0),
                                stop=False,
                            )
                    pend = (c, ets, vt[:, c // SQ, c % SQ, :])
                pc, pets, pv = pend
                for j, ut in enumerate((utA, utB)):
                    nc.tensor.matmul(
                        ut[:65, :],
                        pv[:, ds(j * 65, 65)],
                        pets[j][:],
                        start=False,
                        stop=True,
                    )
                norm_pend = (p, utA, utB)
            emit_normalize(*norm_pend)

            # ---- output projection + residual + LayerNorm ----
            out_sb = big.tile([P, SQ, D], FP32, tag="out")
            wo_sb = wpool.tile([P, KC, D], FP32R, tag="wrhs")
            for k in range(KC):
                nc.sync.dma_start(wo_sb[:, k, :], wo_d[ts(k, P), :])
            # i-outer so each row chunk's LayerNorm starts as soon as its
            # two 512-col halves are projected, instead of after all of them
            for i in range(SQ):
                for n in range(2):
                    pp = psA.tile([P, 512], FP32, tag="mm")
                    for p in range(PAIRS):
                        nc.tensor.matmul(
                            pp[:],
                            ctx_sb[:, p, ts(i, P)],
                            wo_sb[:, p, ds(n * 512, 512)],
                            start=(p == 0),
                            stop=(p == PAIRS - 1),
                        )
                    res = stream.tile([P, 512], FP32R, tag="res")
                    nc.sync.dma_start(res[:], xq_d[ts(i, P), ds(n * 512, 512)])
                    tmp = stream.tile([P, 512], FP32, tag="otmp")
                    nc.vector.tensor_tensor(tmp[:], pp[:], res[:], ALU.add)
                    nc.vector.tensor_tensor(
                        out_sb[:, i, ds(n * 512, 512)],
                        tmp[:],
                        bo_b[:, ds(n * 512, 512)],
                        ALU.add,
                    )
                row = out_sb[:, i, :]
                stats = small.tile([P, 2, 6], FP32, tag="stats")
                nc.vector.bn_stats(stats[:, 0, :], row[:, 0:512])
                nc.vector.bn_stats(stats[:, 1, :], row[:, 512:1024])
                mv = small.tile([P, 2], FP32, tag="mv")
                nc.vector.bn_aggr(mv[:], stats[:])
                std = small.tile([P, 1], FP32, tag="std")
                nc.scalar.activation(
                    out=std[:], in_=mv[:, 1:2], func=AF.Sqrt, bias=eps_t[:], scale=1.0
                )
                rstd = small.tile([P, 1], FP32, tag="rstd")
                nc.vector.reciprocal(out=rstd[:], in_=std[:])
                ytile = wide.tile([P, D], FP32, tag="y")
                nc.vector.tensor_scalar(
                    out=ytile[:],
                    in0=row,
                    scalar1=mv[:, 0:1],
                    scalar2=rstd[:],
                    op0=ALU.subtract,
                    op1=ALU.mult,
                )
                nc.vector.tensor_tensor(ytile[:], ytile[:], gam_b[:], ALU.mult)
                nc.vector.tensor_tensor(ytile[:], ytile[:], bet_b[:], ALU.add)
                nc.sync.dma_start(y_d[ts(i, P), :], ytile[:])

    nc.compile()
    return nc


def get_nc():
    if "nc" not in _NC_CACHE:
        _NC_CACHE["nc"] = build_nc()
    return _NC_CACHE["nc"]


def kernel(
    query,
    key,
    value,
    Wq,
    bq,
    Wk,
    bk,
    Wv,
    bv,
    Wo,
    bo,
    ln_gamma,
    ln_beta,
    _trace=False,
    _trace_cores=None,
):
    query = np.ascontiguousarray(np.asarray(query, dtype=np.float32))
    key = np.ascontiguousarray(np.asarray(key, dtype=np.float32))
    value = np.ascontiguousarray(np.asarray(value, dtype=np.float32))
    shared = {
        "wq": np.ascontiguousarray(np.asarray(Wq, np.float32)),
        "wk": np.ascontiguousarray(np.asarray(Wk, np.float32)),
        "wv": np.ascontiguousarray(np.asarray(Wv, np.float32)),
        "wo": np.ascontiguousarray(np.asarray(Wo, np.float32)),
        "bq": np.ascontiguousarray(np.asarray(bq, np.float32)),
        "bk": np.ascontiguousarray(np.asarray(bk, np.float32)),
        "bv": np.ascontiguousarray(np.asarray(bv, np.float32)),
        "bo": np.ascontiguousarray(np.asarray(bo, np.float32)),
        "gam": np.ascontiguousarray(np.asarray(ln_gamma, np.float32)),
        "bet": np.ascontiguousarray(np.asarray(ln_beta, np.float32)),
        "ident": np.eye(P, dtype=np.float32),
        "ones": np.ones((P, 64), dtype=np.float32),
    }
    in_maps = []
    for c in range(N_CORES):
        b, r = divmod(c, NB)
        rows = slice(r * SL, (r + 1) * SL)
        m = dict(shared)
        m["xq"] = np.ascontiguousarray(query[b, rows, :])
        m["xk"] = np.ascontiguousarray(key[b])
        m["xv"] = np.ascontiguousarray(value[b])
        in_maps.append(m)

    nc = get_nc()
    res = run_bass_kernel_spmd(
        nc,
        in_maps,
        list(range(N_CORES)),
        trace=_trace,
        trace_cores=_trace_cores,
    )
    out = np.empty((B, S, D), dtype=np.float32)
    for c in range(N_CORES):
        b, r = divmod(c, NB)
        out[b, r * SL : (r + 1) * SL, :] = res.results[c]["y"]
    if _trace:
        return out, res
    return out



# revision 8
# speedup vs baseline: 1.2651x; 1.2651x over previous
"""Multi-head attention + residual + LayerNorm on 8 Trainium2 NeuronCores.

Sharding: core c handles batch b = c//4 and query-row quarter g = c%4
(rows 512g..512g+512 of S=2048) with ALL 16 heads for attention, but
projects K/V only for its own 4 heads (g's quarter of the head dim);
the full K^T / V_aug are assembled with per-key-block AllGathers over
the 4-core batch group, which run on the dedicated CC cores and overlap
the remaining projections.  All matmul operands are bf16 (hosts casts);
PSUM accumulation stays fp32, so the projections/attention keep ~1e-3
accuracy (gate is 2e-2).

Per core:
  - PE-transpose bf16 x tiles (via identity) for xk / xv / xq
  - K^T own-head block [256, 512] per key block -> AllGather -> [1024, 512]
  - V own-head block + bias + pair-augmented ones cols [512, 2, 130]
    -> AllGather
  - Q^T all heads [128, 8, 512]
  - attention per head pair p: scores S^T = K_h Q_h^T with the two
    dk=64 heads PAIRED via PE tile_position (rows 0-63 / 64-127 run
    concurrently), one batched exp ACT per chunk [128, 1024] -> bf16,
    U^T accumulation with lhsT = V_aug (ones col makes row 64 the
    softmax denominators), reciprocal+PE-broadcast normalize -> ctx^T
  - out = ctx @ Wo + bo + residual -> LayerNorm -> y rows [512, 1024]
"""

import sys

if "/opt/trn_rl_repo" not in sys.path:
    sys.path.insert(0, "/opt/trn_rl_repo")

import numpy as np

import concourse.bacc as bacc
import concourse.bass as bass
import concourse.mybir as mybir
import concourse.tile as tile
from concourse.bass import ds, ts
from concourse.bass_utils import run_bass_kernel_spmd

FP32R = mybir.dt.float32r
FP32 = mybir.dt.float32
BF16 = mybir.dt.bfloat16
AF = mybir.ActivationFunctionType
ALU = mybir.AluOpType

N_CORES = 8
B = 2
S = 2048
D = 1024
H = 16
DK = 64
P = 128

SL = S // 4  # 512 local query rows per core
KC = D // P  # 8 contraction chunks over d_model
SQ = SL // P  # 4 x 128-row subchunks per 512 block
CH = S // P  # 16 sk chunks
PAIRS = H // 2  # 8 head pairs
NB = 4  # 512-row key blocks
OWN = D // 4  # 256 own-head output dims per core
EPS = 1e-5
GROUPS = [[0, 1, 2, 3], [4, 5, 6, 7]]

_NC_CACHE = {}


def build_nc():
    nc = bacc.Bacc(num_devices=N_CORES)

    xq_d = nc.dram_tensor("xq", [SL, D], BF16, kind="ExternalInput")
    xqr_d = nc.dram_tensor("xqr", [SL, D], FP32, kind="ExternalInput")
    xk_d = nc.dram_tensor("xk", [S, D], BF16, kind="ExternalInput")
    xv_d = nc.dram_tensor("xv", [S, D], BF16, kind="ExternalInput")
    wq_d = nc.dram_tensor("wq", [D, D], BF16, kind="ExternalInput")
    wk_d = nc.dram_tensor("wk", [D, OWN], BF16, kind="ExternalInput")
    wv_d = nc.dram_tensor("wv", [D, OWN], BF16, kind="ExternalInput")
    wo_d = nc.dram_tensor("wo", [D, D], BF16, kind="ExternalInput")
    bq_d = nc.dram_tensor("bq", [P, KC], FP32, kind="ExternalInput")
    bk_d = nc.dram_tensor("bk", [P, 2], FP32, kind="ExternalInput")
    bv_d = nc.dram_tensor("bv", [OWN], FP32, kind="ExternalInput")
    bo_d = nc.dram_tensor("bo", [D], FP32, kind="ExternalInput")
    gam_d = nc.dram_tensor("gam", [D], FP32, kind="ExternalInput")
    bet_d = nc.dram_tensor("bet", [D], FP32, kind="ExternalInput")
    ident_d = nc.dram_tensor("ident", [P, P], BF16, kind="ExternalInput")
    ones_d = nc.dram_tensor("ones", [P, 64], FP32R, kind="ExternalInput")

    y_d = nc.dram_tensor("y", [SL, D], FP32, kind="ExternalOutput")

    # collective bounce buffers (DRAM)
    kag_in = nc.dram_tensor("kag_in", [NB, OWN, SL], BF16)
    kag_out = nc.dram_tensor("kag_out", [NB, 4, OWN, SL], BF16)
    vag_in = nc.dram_tensor("vag_in", [NB, SL, 2, 130], BF16)
    vag_out = nc.dram_tensor("vag_out", [NB, 4, SL, 2, 130], BF16)

    with tile.TileContext(nc) as tc:
        with (
            tc.tile_pool(name="consts", bufs=1) as consts,
            tc.tile_pool(name="wpool", bufs=1) as wpool,
            tc.tile_pool(name="xnp", bufs=3) as xnp,
            tc.tile_pool(name="xtp", bufs=2) as xtp,
            tc.tile_pool(name="big", bufs=1) as big,
            tc.tile_pool(name="stream", bufs=3) as stream,
            tc.tile_pool(name="kttp", bufs=2) as kttp,
            tc.tile_pool(name="vat", bufs=2) as vatp,
            tc.tile_pool(name="etp", bufs=3) as etp,
            tc.tile_pool(name="small", bufs=2) as small,
            tc.tile_pool(name="wide", bufs=1) as wide,
            tc.tile_pool(name="sc", bufs=2, space="PSUM") as scp,
            tc.tile_pool(name="acc", bufs=1, space="PSUM") as accp,
            tc.tile_pool(name="bc", bufs=1, space="PSUM") as bcp,
        ):
            # ---- constants ----
            ident = consts.tile([P, P], BF16)
            nc.sync.dma_start(ident[:], ident_d[:])
            ones64 = consts.tile([P, 64], FP32R)
            nc.sync.dma_start(ones64[:], ones_d[:])
            ones_bf = consts.tile([P, 8], BF16)
            nc.vector.memset(ones_bf[:], 1.0)
            bq_sb = consts.tile([P, KC], FP32)
            nc.sync.dma_start(bq_sb[:], bq_d[:])
            bk_sb = consts.tile([P, 2], FP32)
            nc.sync.dma_start(bk_sb[:], bk_d[:])

            def bcast_load(src, tag, n):
                t = consts.tile([P, n], FP32, tag=tag)
                ap = bass.AP(tensor=src, offset=0, ap=[[0, P], [1, n]])
                nc.gpsimd.dma_start(out=t[:], in_=ap)
                return t

            bv_b = bcast_load(bv_d, "bv_b", OWN)
            bo_b = bcast_load(bo_d, "bo_b", D)
            gam_b = bcast_load(gam_d, "gam_b", D)
            bet_b = bcast_load(bet_d, "bet_b", D)
            eps_t = consts.tile([P, 1], FP32)
            nc.vector.memset(eps_t[:], EPS)

            # ---- weights (bf16) ----
            wq_sb = wpool.tile([P, KC, D], BF16, tag="wq")
            for k in range(KC):
                nc.sync.dma_start(wq_sb[:, k, :], wq_d[ts(k, P), :])
            wk_sb = wpool.tile([P, KC, OWN], BF16, tag="wk")
            for k in range(KC):
                nc.sync.dma_start(wk_sb[:, k, :], wk_d[ts(k, P), :])
            wv_sb = wpool.tile([P, KC, OWN], BF16, tag="wv")
            for k in range(KC):
                nc.sync.dma_start(wv_sb[:, k, :], wv_d[ts(k, P), :])
            wo_sb = wpool.tile([P, KC, D], BF16, tag="wo")
            for k in range(KC):
                nc.sync.dma_start(wo_sb[:, k, :], wo_d[ts(k, P), :])

            def load_xT(x_d, row0):
                """x rows [row0:row0+512] (bf16) -> x^T SBUF [128, KC, 512].

                Transposes go through PSUM in [128, 1024] batches; the
                PSUM->SBUF bf16 copies run on the Scalar engine (Identity),
                which is idle during the projection phase.
                """
                xT = xtp.tile([P, KC, SL], BF16, tag="xT")
                for i in range(SQ):
                    xt = xnp.tile([P, D], BF16, tag="xnat")
                    nc.sync.dma_start(xt[:], x_d[ds(row0 + i * P, P), :])
                    pt = scp.tile([P, 1024], BF16, tag="sc")
                    for j in range(KC):
                        nc.tensor.transpose(pt[:, ts(j, P)], xt[:, ts(j, P)], ident[:])
                    nc.scalar.activation(
                        out=xT[:, :, ts(i, P)],
                        in_=pt[:].rearrange("q (k s) -> q k s", k=KC),
                        func=AF.Identity,
                    )
                return xT

            # ---- K^T own heads, per key block -> AllGather ----
            for blk in range(NB):
                xkT = load_xT(xk_d, blk * SL)
                kev = stream.tile([P, 2, SL], BF16, tag="kev")
                for m in range(2):
                    pp = scp.tile([P, 1024], FP32, tag="sc")
                    for k in range(KC):
                        nc.tensor.matmul(
                            pp[:, 0:SL],
                            wk_sb[:, k, ts(m, P)],
                            xkT[:, k, :],
                            start=(k == 0),
                            stop=(k == KC - 1),
                        )
                    nc.vector.tensor_scalar(
                        out=kev[:, m, :],
                        in0=pp[:, 0:SL],
                        scalar1=bk_sb[:, m : m + 1],
                        scalar2=None,
                        op0=ALU.add,
                    )
                nc.sync.dma_start(
                    kag_in[blk].rearrange("(m q) s -> q m s", q=P), kev[:]
                )
                nc.gpsimd.collective_compute(
                    "AllGather",
                    ALU.bypass,
                    replica_groups=GROUPS,
                    ins=[kag_in[blk]],
                    outs=[kag_out[blk]],
                )

            # ---- V own heads, per key block -> AllGather ----
            for blk in range(NB):
                xvT = load_xT(xv_d, blk * SL)
                vtmp = stream.tile([P, SQ, 2, 130], BF16, tag="vtmp")
                for i in range(SQ):
                    pp = scp.tile([P, 1024], FP32, tag="sc")
                    for k in range(KC):
                        nc.tensor.matmul(
                            pp[:, 0:OWN],
                            xvT[:, k, ts(i, P)],
                            wv_sb[:, k, :],
                            start=(k == 0),
                            stop=(k == KC - 1),
                        )
                    vdst = vtmp[:, i].rearrange("q pl (j e) -> q pl j e", e=65)
                    nc.vector.tensor_tensor(
                        vdst[:, :, :, 0:64],
                        pp[:, 0:OWN].rearrange("q (pl j e) -> q pl j e", pl=2, j=2),
                        bv_b[:].rearrange("q (pl j e) -> q pl j e", pl=2, j=2),
                        ALU.add,
                    )
                    nc.vector.tensor_copy(
                        vdst[:, :, :, 64:65],
                        ones_bf[:, 0:4, None].rearrange("q (pl j) o -> q pl j o", pl=2),
                    )
                nc.sync.dma_start(
                    vag_in[blk].rearrange("(i q) pl e -> q i pl e", q=P), vtmp[:]
                )
                nc.gpsimd.collective_compute(
                    "AllGather",
                    ALU.bypass,
                    replica_groups=GROUPS,
                    ins=[vag_in[blk]],
                    outs=[vag_out[blk]],
                )

            # ---- Q^T all heads (own 512 rows) ----
            xqT = load_xT(xq_d, 0)
            qt_sb = big.tile([P, KC, SL], BF16, tag="qt")
            for m in range(KC):
                pp = scp.tile([P, 1024], FP32, tag="sc")
                for k in range(KC):
                    nc.tensor.matmul(
                        pp[:, 0:SL],
                        wq_sb[:, k, ts(m, P)],
                        xqT[:, k, :],
                        start=(k == 0),
                        stop=(k == KC - 1),
                    )
                nc.vector.tensor_scalar(
                    out=qt_sb[:, m, :],
                    in0=pp[:, 0:SL],
                    scalar1=bq_sb[:, m : m + 1],
                    scalar2=None,
                    op0=ALU.add,
                )

            # ---- attention ----
            ctx_sb = big.tile([P, PAIRS, SL], BF16, tag="ctx")

            def emit_normalize(np_, uA, uB):
                # rows 0..63 of ut / row 64 -> ctx_sb[:, np_, :]
                for j, ut in enumerate((uA, uB)):
                    rec = small.tile([P, SL], FP32R, tag="rec")
                    with nc.allow_low_precision(
                        reason="float32r is bit-identical to float32 in SBUF"
                    ):
                        nc.vector.reciprocal(out=rec[64:65, :], in_=ut[64:65, :])
                    bc = bcp.tile([P, SL], FP32, tag="bc")
                    nc.tensor.matmul(
                        bc[0:64, :],
                        ones64[64:65, :],
                        rec[64:65, :],
                        start=True,
                        stop=True,
                    )
                    bc_sb = small.tile([P, SL], FP32, tag="bcs")
                    nc.vector.tensor_copy(bc_sb[0:64, :], bc[0:64, :])
                    if j == 0:
                        nc.vector.tensor_tensor(
                            ctx_sb[0:64, np_, :], ut[0:64, :], bc_sb[0:64, :], ALU.mult
                        )
                    else:
                        ctmp = small.tile([P, SL], BF16, tag="ctmp")
                        nc.vector.tensor_tensor(
                            ctmp[0:64, :], ut[0:64, :], bc_sb[0:64, :], ALU.mult
                        )
                        # partition shift 0-63 -> 64-127 via SBUF-SBUF DMA
                        nc.sync.dma_start(ctx_sb[64:128, np_, :], ctmp[0:64, :])

            def prefetch_pair(pp_):
                kt = kttp.tile([P, CH, P], BF16, tag="ktt", name=f"kt_{pp_}")
                vt = vatp.tile([P, CH, 130], BF16, tag="vat", name=f"vt_{pp_}")
                grp, half = pp_ // 2, pp_ % 2
                for kb in range(NB):
                    nc.sync.dma_start(
                        kt[:, ds(kb * SQ, SQ), :].rearrange("q c s -> q (c s)"),
                        kag_out[kb, grp, ds(half * P, P), :],
                    )
                    nc.sync.dma_start(
                        vt[:, ds(kb * SQ, SQ), :],
                        vag_out[kb, grp]
                        .rearrange("(c q) pl e -> q c pl e", q=P)[:, :, half, :],
                    )
                return kt, vt

            tiles = {0: prefetch_pair(0)}
            norm_pend = None
            for p in range(PAIRS):
                kt, vt = tiles.pop(p)
                utA = utB = None
                pend = None
                for c in range(CH):
                    st = scp.tile([P, 1024], FP32, tag="sc")
                    for j in range(2):
                        nc.tensor.matmul(
                            st[:, ds(j * SL, SL)],
                            kt[ds(j * 64, 64), c, :],
                            qt_sb[ds(j * 64, 64), p, :],
                            start=True,
                            stop=True,
                            tile_position=(j * 64, 0),
                        )
                    et = etp.tile([P, 1024], BF16, tag="et")
                    nc.scalar.activation(out=et[:], in_=st[:], func=AF.Exp, scale=0.125)
                    if c == 0:
                        # previous pair's normalize: emitted before this pair's
                        # accumulators are (re)allocated so the single-buffered
                        # pool sees the WAR (in-order PE stays live)
                        if norm_pend is not None:
                            emit_normalize(*norm_pend)
                            norm_pend = None
                        utA = accp.tile([P, SL], FP32, tag="accA")
                        utB = accp.tile([P, SL], FP32, tag="accB")
                    if c == 4 and p + 1 < PAIRS:
                        tiles[p + 1] = prefetch_pair(p + 1)
                    if pend is not None:
                        pc, pet, pv = pend
                        for j, ut in enumerate((utA, utB)):
                            nc.tensor.matmul(
                                ut[:65, :],
                                pv[:, ds(j * 65, 65)],
                                pet[:, ds(j * SL, SL)],
                                start=(pc == 0),
                                stop=False,
                            )
                    pend = (c, et, vt[:, c, :])
                pc, pet, pv = pend
                for j, ut in enumerate((utA, utB)):
                    nc.tensor.matmul(
                        ut[:65, :],
                        pv[:, ds(j * 65, 65)],
                        pet[:, ds(j * SL, SL)],
                        start=False,
                        stop=True,
                    )
                norm_pend = (p, utA, utB)
            emit_normalize(*norm_pend)

            # ---- output projection + residual + LayerNorm ----
            out_sb = big.tile([P, SQ, D], FP32, tag="out")
            for i in range(SQ):
                for n in range(2):
                    pp = scp.tile([P, 1024], FP32, tag="sc")
                    for p in range(PAIRS):
                        nc.tensor.matmul(
                            pp[:, 0:512],
                            ctx_sb[:, p, ts(i, P)],
                            wo_sb[:, p, ds(n * 512, 512)],
                            start=(p == 0),
                            stop=(p == PAIRS - 1),
                        )
                    res = stream.tile([P, 512], FP32, tag="res")
                    nc.sync.dma_start(res[:], xqr_d[ts(i, P), ds(n * 512, 512)])
                    tmp = stream.tile([P, 512], FP32, tag="otmp")
                    nc.vector.tensor_tensor(tmp[:], pp[:, 0:512], res[:], ALU.add)
                    nc.vector.tensor_tensor(
                        out_sb[:, i, ds(n * 512, 512)],
                        tmp[:],
                        bo_b[:, ds(n * 512, 512)],
                        ALU.add,
                    )
                row = out_sb[:, i, :]
                stats = small.tile([P, 2, 6], FP32, tag="stats")
                nc.vector.bn_stats(stats[:, 0, :], row[:, 0:512])
                nc.vector.bn_stats(stats[:, 1, :], row[:, 512:1024])
                mv = small.tile([P, 2], FP32, tag="mv")
                nc.vector.bn_aggr(mv[:], stats[:])
                std = small.tile([P, 1], FP32, tag="std")
                nc.scalar.activation(
                    out=std[:], in_=mv[:, 1:2], func=AF.Sqrt, bias=eps_t[:], scale=1.0
                )
                rstd = small.tile([P, 1], FP32, tag="rstd")
                nc.vector.reciprocal(out=rstd[:], in_=std[:])
                ytile = wide.tile([P, D], FP32, tag="y")
                nc.vector.tensor_scalar(
                    out=ytile[:],
                    in0=row,
                    scalar1=mv[:, 0:1],
                    scalar2=rstd[:],
                    op0=ALU.subtract,
                    op1=ALU.mult,
                )
                nc.vector.tensor_tensor(ytile[:], ytile[:], gam_b[:], ALU.mult)
                nc.vector.tensor_tensor(ytile[:], ytile[:], bet_b[:], ALU.add)
                nc.sync.dma_start(y_d[ts(i, P), :], ytile[:])

    nc.compile()
    return nc


def get_nc():
    if "nc" not in _NC_CACHE:
        _NC_CACHE["nc"] = build_nc()
    return _NC_CACHE["nc"]


def kernel(
    query,
    key,
    value,
    Wq,
    bq,
    Wk,
    bk,
    Wv,
    bv,
    Wo,
    bo,
    ln_gamma,
    ln_beta,
    _trace=False,
    _trace_cores=None,
):
    import ml_dtypes

    BF = ml_dtypes.bfloat16
    query = np.ascontiguousarray(np.asarray(query, dtype=np.float32))
    key_bf = np.ascontiguousarray(np.asarray(key, np.float32)).astype(BF)
    value_bf = np.ascontiguousarray(np.asarray(value, np.float32)).astype(BF)
    query_bf = query.astype(BF)
    Wq = np.asarray(Wq, np.float32)
    Wk = np.asarray(Wk, np.float32)
    Wv = np.asarray(Wv, np.float32)
    Wo = np.asarray(Wo, np.float32)
    bq_r = np.ascontiguousarray(
        np.asarray(bq, np.float32).reshape(KC, P).T
    )  # [P, KC] with bias for dim 128k+p at [p, k]
    bk_f = np.asarray(bk, np.float32)
    bv_f = np.asarray(bv, np.float32)
    shared = {
        "wq": np.ascontiguousarray(Wq.astype(BF)),
        "wo": np.ascontiguousarray(Wo.astype(BF)),
        "bq": bq_r,
        "bo": np.ascontiguousarray(np.asarray(bo, np.float32)),
        "gam": np.ascontiguousarray(np.asarray(ln_gamma, np.float32)),
        "bet": np.ascontiguousarray(np.asarray(ln_beta, np.float32)),
        "ident": np.eye(P, dtype=BF),
        "ones": np.ones((P, 64), dtype=np.float32),
    }
    in_maps = []
    for c in range(N_CORES):
        b, g = divmod(c, NB)
        rows = slice(g * SL, (g + 1) * SL)
        cols = slice(g * OWN, (g + 1) * OWN)
        m = dict(shared)
        m["xq"] = np.ascontiguousarray(query_bf[b, rows, :])
        m["xqr"] = np.ascontiguousarray(query[b, rows, :])
        m["xk"] = key_bf[b]
        m["xv"] = value_bf[b]
        m["wk"] = np.ascontiguousarray(Wk[:, cols].astype(BF))
        m["wv"] = np.ascontiguousarray(Wv[:, cols].astype(BF))
        m["bk"] = np.ascontiguousarray(bk_f[cols].reshape(2, P).T)
        m["bv"] = np.ascontiguousarray(bv_f[cols])
        in_maps.append(m)

    nc = get_nc()
    res = run_bass_kernel_spmd(
        nc,
        in_maps,
        list(range(N_CORES)),
        trace=_trace,
        trace_cores=_trace_cores,
    )
    out = np.empty((B, S, D), dtype=np.float32)
    for c in range(N_CORES):
        b, g = divmod(c, NB)
        out[b, g * SL : (g + 1) * SL, :] = res.results[c]["y"]
    if _trace:
        return out, res
    return out


# revision 14
# speedup vs baseline: 1.4654x; 1.1583x over previous
"""Multi-head attention + residual + LayerNorm on 8 Trainium2 NeuronCores.

Sharding: core c handles batch b = c//4 and query-row quarter g = c%4
(rows 512g..512g+512 of S=2048) with ALL 16 heads for attention, but
projects K/V only for its own 4 heads (g's quarter of the head dim);
the full K^T / V_aug are assembled with per-key-block AllGathers over
the 4-core batch group, which run on the dedicated CC cores and overlap
the remaining projections.  All matmul operands are bf16 (hosts casts);
PSUM accumulation stays fp32, so the projections/attention keep ~1e-3
accuracy (gate is 2e-2).

Per core:
  - PE-transpose bf16 x tiles (via identity) for xk / xv / xq
  - K^T own-head block [256, 512] per key block -> AllGather -> [1024, 512]
  - V own-head block + bias + pair-augmented ones cols [512, 2, 130]
    -> AllGather
  - Q^T all heads [128, 8, 512]
  - attention per head pair p: scores S^T = K_h Q_h^T with the two
    dk=64 heads PAIRED via PE tile_position (rows 0-63 / 64-127 run
    concurrently), one batched exp ACT per chunk [128, 1024] -> bf16,
    U^T accumulation with lhsT = V_aug (ones col makes row 64 the
    softmax denominators), reciprocal+PE-broadcast normalize -> ctx^T
  - out = ctx @ Wo + bo + residual -> LayerNorm -> y rows [512, 1024]
"""

import sys

if "/opt/trn_rl_repo" not in sys.path:
    sys.path.insert(0, "/opt/trn_rl_repo")

import numpy as np

import concourse.bacc as bacc
import concourse.bass as bass
import concourse.mybir as mybir
import concourse.tile as tile
from concourse.bass import ds, ts
from concourse.bass_utils import run_bass_kernel_spmd

FP32R = mybir.dt.float32r
FP32 = mybir.dt.float32
BF16 = mybir.dt.bfloat16
AF = mybir.ActivationFunctionType
ALU = mybir.AluOpType

N_CORES = 8
B = 2
S = 2048
D = 1024
H = 16
DK = 64
P = 128

SL = S // 4  # 512 local query rows per core
KC = D // P  # 8 contraction chunks over d_model
SQ = SL // P  # 4 x 128-row subchunks per 512 block
CH = S // P  # 16 sk chunks
PAIRS = H // 2  # 8 head pairs
NB = 4  # 512-row key blocks
OWN = D // 4  # 256 own-head output dims per core
EPS = 1e-5
GROUPS = [[0, 1, 2, 3], [4, 5, 6, 7]]

_NC_CACHE = {}


def build_nc():
    nc = bacc.Bacc(num_devices=N_CORES)

    xq_d = nc.dram_tensor("xq", [SL, D], BF16, kind="ExternalInput")
    xqr_d = nc.dram_tensor("xqr", [SL, D], FP32, kind="ExternalInput")
    xk_d = nc.dram_tensor("xk", [S, D], BF16, kind="ExternalInput")
    xv_d = nc.dram_tensor("xv", [S, D], BF16, kind="ExternalInput")
    wq_d = nc.dram_tensor("wq", [D, D], BF16, kind="ExternalInput")
    wk_d = nc.dram_tensor("wk", [D, OWN], BF16, kind="ExternalInput")
    wv_d = nc.dram_tensor("wv", [D, OWN], BF16, kind="ExternalInput")
    wo_d = nc.dram_tensor("wo", [D, D], BF16, kind="ExternalInput")
    bq_d = nc.dram_tensor("bq", [P, KC], FP32, kind="ExternalInput")
    bk_d = nc.dram_tensor("bk", [P, 2], FP32, kind="ExternalInput")
    bv_d = nc.dram_tensor("bv", [OWN], FP32, kind="ExternalInput")
    bo_d = nc.dram_tensor("bo", [D], FP32, kind="ExternalInput")
    gam_d = nc.dram_tensor("gam", [D], FP32, kind="ExternalInput")
    bet_d = nc.dram_tensor("bet", [D], FP32, kind="ExternalInput")
    ident_d = nc.dram_tensor("ident", [P, P], BF16, kind="ExternalInput")
    ones_d = nc.dram_tensor("ones", [P, 64], FP32R, kind="ExternalInput")

    y_d = nc.dram_tensor("y", [SL, D], FP32, kind="ExternalOutput")

    # collective bounce buffers (DRAM)
    kag_in = nc.dram_tensor("kag_in", [NB, OWN, SL], BF16)
    kag_out = nc.dram_tensor("kag_out", [NB, 4, OWN, SL], BF16)
    vag_in = nc.dram_tensor("vag_in", [NB, SL, 2, 130], BF16)
    vag_out = nc.dram_tensor("vag_out", [NB, 4, SL, 2, 130], BF16)

    with tile.TileContext(nc) as tc:
        with (
            tc.tile_pool(name="consts", bufs=1) as consts,
            tc.tile_pool(name="wpool", bufs=1) as wpool,
            tc.tile_pool(name="xnp", bufs=3) as xnp,
            tc.tile_pool(name="xtp", bufs=2) as xtp,
            tc.tile_pool(name="big", bufs=1) as big,
            tc.tile_pool(name="stream", bufs=3) as stream,
            tc.tile_pool(name="kttp", bufs=2) as kttp,
            tc.tile_pool(name="vat", bufs=2) as vatp,
            tc.tile_pool(name="etp", bufs=3) as etp,
            tc.tile_pool(name="small", bufs=2) as small,
            tc.tile_pool(name="wide", bufs=1) as wide,
            tc.tile_pool(name="sc", bufs=2, space="PSUM") as scp,
            tc.tile_pool(name="acc", bufs=1, space="PSUM") as accp,
            tc.tile_pool(name="bc", bufs=1, space="PSUM") as bcp,
        ):
            # ---- constants ----
            ident = consts.tile([P, P], BF16)
            nc.sync.dma_start(ident[:], ident_d[:])
            ones64 = consts.tile([P, 64], FP32R)
            nc.sync.dma_start(ones64[:], ones_d[:])
            ones_bf = consts.tile([P, 8], BF16)
            nc.vector.memset(ones_bf[:], 1.0)
            bq_sb = consts.tile([P, KC], FP32)
            nc.sync.dma_start(bq_sb[:], bq_d[:])
            bk_sb = consts.tile([P, 2], FP32)
            nc.sync.dma_start(bk_sb[:], bk_d[:])

            def bcast_load(src, tag, n):
                t = consts.tile([P, n], FP32, tag=tag)
                ap = bass.AP(tensor=src, offset=0, ap=[[0, P], [1, n]])
                nc.gpsimd.dma_start(out=t[:], in_=ap)
                return t

            bv_b = bcast_load(bv_d, "bv_b", OWN)
            bo_b = bcast_load(bo_d, "bo_b", D)
            gam_b = bcast_load(gam_d, "gam_b", D)
            bet_b = bcast_load(bet_d, "bet_b", D)
            eps_t = consts.tile([P, 1], FP32)
            nc.vector.memset(eps_t[:], EPS)

            # ---- weights (bf16), in first-use order: wk, wv, wq, wo ----
            wk_sb = wpool.tile([P, KC, OWN], BF16, tag="wk")
            for k in range(KC):
                nc.sync.dma_start(wk_sb[:, k, :], wk_d[ts(k, P), :])
            wv_sb = wpool.tile([P, KC, OWN], BF16, tag="wv")
            for k in range(KC):
                nc.sync.dma_start(wv_sb[:, k, :], wv_d[ts(k, P), :])
            wq_sb = wpool.tile([P, KC, D], BF16, tag="wq")
            for k in range(KC):
                nc.sync.dma_start(wq_sb[:, k, :], wq_d[ts(k, P), :])
            wo_sb = wpool.tile([P, KC, D], BF16, tag="wo")
            for k in range(KC):
                nc.scalar.dma_start(wo_sb[:, k, :], wo_d[ts(k, P), :])

            def load_xT(x_d, row0):
                """x rows [row0:row0+512] (bf16) -> x^T SBUF [128, KC, 512].

                Transposes go through PSUM in [128, 1024] batches; the
                PSUM->SBUF bf16 copies run on the Scalar engine (Identity),
                which is idle during the projection phase.
                """
                xT = xtp.tile([P, KC, SL], BF16, tag="xT")
                for i in range(SQ):
                    xt = xnp.tile([P, D], BF16, tag="xnat")
                    nc.sync.dma_start(xt[:], x_d[ds(row0 + i * P, P), :])
                    pt = scp.tile([P, 1024], BF16, tag="sc")
                    for j in range(KC):
                        nc.tensor.transpose(pt[:, ts(j, P)], xt[:, ts(j, P)], ident[:])
                    nc.scalar.activation(
                        out=xT[:, :, ts(i, P)],
                        in_=pt[:].rearrange("q (k s) -> q k s", k=KC),
                        func=AF.Identity,
                    )
                return xT

            # ---- K^T own heads, per key block -> AllGather ----
            for blk in range(NB):
                xkT = load_xT(xk_d, blk * SL)
                kev = stream.tile([P, 2, SL], BF16, tag="kev")
                for m in range(2):
                    pp = scp.tile([P, 1024], FP32, tag="sc")
                    for k in range(KC):
                        nc.tensor.matmul(
                            pp[:, 0:SL],
                            wk_sb[:, k, ts(m, P)],
                            xkT[:, k, :],
                            start=(k == 0),
                            stop=(k == KC - 1),
                        )
                    nc.vector.tensor_scalar(
                        out=kev[:, m, :],
                        in0=pp[:, 0:SL],
                        scalar1=bk_sb[:, m : m + 1],
                        scalar2=None,
                        op0=ALU.add,
                    )
                nc.scalar.dma_start(
                    kag_in[blk].rearrange("(m q) s -> q m s", q=P), kev[:]
                )
                nc.gpsimd.collective_compute(
                    "AllGather",
                    ALU.bypass,
                    replica_groups=GROUPS,
                    ins=[kag_in[blk]],
                    outs=[kag_out[blk]],
                )

            # ---- V own heads, per key block -> AllGather ----
            for blk in range(NB):
                xvT = load_xT(xv_d, blk * SL)
                vtmp = stream.tile([P, SQ, 2, 130], BF16, tag="vtmp")
                for i in range(SQ):
                    pp = scp.tile([P, 1024], FP32, tag="sc")
                    for k in range(KC):
                        nc.tensor.matmul(
                            pp[:, 0:OWN],
                            xvT[:, k, ts(i, P)],
                            wv_sb[:, k, :],
                            start=(k == 0),
                            stop=(k == KC - 1),
                        )
                    vdst = vtmp[:, i].rearrange("q pl (j e) -> q pl j e", e=65)
                    nc.vector.tensor_tensor(
                        vdst[:, :, :, 0:64],
                        pp[:, 0:OWN].rearrange("q (pl j e) -> q pl j e", pl=2, j=2),
                        bv_b[:].rearrange("q (pl j e) -> q pl j e", pl=2, j=2),
                        ALU.add,
                    )
                    nc.vector.tensor_copy(
                        vdst[:, :, :, 64:65],
                        ones_bf[:, 0:4, None].rearrange("q (pl j) o -> q pl j o", pl=2),
                    )
                nc.scalar.dma_start(
                    vag_in[blk].rearrange("(i q) pl e -> q i pl e", q=P), vtmp[:]
                )
                nc.gpsimd.collective_compute(
                    "AllGather",
                    ALU.bypass,
                    replica_groups=GROUPS,
                    ins=[vag_in[blk]],
                    outs=[vag_out[blk]],
                )

            # ---- Q^T all heads (own 512 rows) ----
            xqT = load_xT(xq_d, 0)
            qt_sb = big.tile([P, KC, SL], BF16, tag="qt")
            for m in range(KC):
                pp = scp.tile([P, 1024], FP32, tag="sc")
                for k in range(KC):
                    nc.tensor.matmul(
                        pp[:, 0:SL],
                        wq_sb[:, k, ts(m, P)],
                        xqT[:, k, :],
                        start=(k == 0),
                        stop=(k == KC - 1),
                    )
                nc.vector.tensor_scalar(
                    out=qt_sb[:, m, :],
                    in0=pp[:, 0:SL],
                    scalar1=bq_sb[:, m : m + 1],
                    scalar2=None,
                    op0=ALU.add,
                )

            # ---- attention ----
            ctx_sb = big.tile([P, PAIRS, SL], BF16, tag="ctx")

            def emit_normalize(np_, uA, uB):
                # rows 0..63 of ut / row 64 -> ctx_sb[:, np_, :].  The raw
                # denominator row is PE-broadcast to 64 partitions FIRST so
                # the DVE reciprocal runs on 64 lanes, not 1 (3.2us -> 0.7us).
                for j, ut in enumerate((uA, uB)):
                    ut_sb = small.tile([P, SL], FP32R, tag="utsb")
                    nc.vector.tensor_copy(ut_sb[0:65, :], ut[0:65, :])
                    bc = bcp.tile([P, SL], FP32, tag="bc")
                    nc.tensor.matmul(
                        bc[0:64, :],
                        ones64[64:65, :],
                        ut_sb[64:65, :],
                        start=True,
                        stop=True,
                    )
                    rec_sb = small.tile([P, SL], FP32, tag="bcs")
                    with nc.allow_low_precision(
                        reason="fp32 reciprocal of softmax denominators"
                    ):
                        nc.vector.reciprocal(out=rec_sb[0:64, :], in_=bc[0:64, :])
                    if j == 0:
                        nc.vector.tensor_tensor(
                            ctx_sb[0:64, np_, :],
                            ut_sb[0:64, :],
                            rec_sb[0:64, :],
                            ALU.mult,
                        )
                    else:
                        ctmp = small.tile([P, SL], BF16, tag="ctmp")
                        nc.vector.tensor_tensor(
                            ctmp[0:64, :], ut_sb[0:64, :], rec_sb[0:64, :], ALU.mult
                        )
                        # partition shift 0-63 -> 64-127 via SBUF-SBUF DMA
                        nc.sync.dma_start(ctx_sb[64:128, np_, :], ctmp[0:64, :])

            def prefetch_pair(pp_):
                kt = kttp.tile([P, CH, P], BF16, tag="ktt", name=f"kt_{pp_}")
                vt = vatp.tile([P, CH, 130], BF16, tag="vat", name=f"vt_{pp_}")
                grp, half = pp_ // 2, pp_ % 2
                for kb in range(NB):
                    nc.sync.dma_start(
                        kt[:, ds(kb * SQ, SQ), :].rearrange("q c s -> q (c s)"),
                        kag_out[kb, grp, ds(half * P, P), :],
                    )
                    nc.sync.dma_start(
                        vt[:, ds(kb * SQ, SQ), :],
                        vag_out[kb, grp]
                        .rearrange("(c q) pl e -> q c pl e", q=P)[:, :, half, :],
                    )
                return kt, vt

            tiles = {0: prefetch_pair(0)}
            norm_pend = None
            for p in range(PAIRS):
                kt, vt = tiles.pop(p)
                utA = utB = None
                pend = None
                for c in range(CH):
                    st = scp.tile([P, 1024], FP32, tag="sc")
                    for j in range(2):
                        nc.tensor.matmul(
                            st[:, ds(j * SL, SL)],
                            kt[ds(j * 64, 64), c, :],
                            qt_sb[ds(j * 64, 64), p, :],
                            start=True,
                            stop=True,
                            tile_position=(j * 64, 0),
                        )
                    et = etp.tile([P, 1024], BF16, tag="et")
                    nc.scalar.activation(out=et[:], in_=st[:], func=AF.Exp, scale=0.125)
                    if c == 0:
                        # previous pair's normalize: emitted before this pair's
                        # accumulators are (re)allocated so the single-buffered
                        # pool sees the WAR (in-order PE stays live)
                        if norm_pend is not None:
                            emit_normalize(*norm_pend)
                            norm_pend = None
                        utA = accp.tile([P, SL], FP32, tag="accA")
                        utB = accp.tile([P, SL], FP32, tag="accB")
                    if c == 4 and p + 1 < PAIRS:
                        tiles[p + 1] = prefetch_pair(p + 1)
                    if pend is not None:
                        pc, pet, pv = pend
                        for j, ut in enumerate((utA, utB)):
                            nc.tensor.matmul(
                                ut[:65, :],
                                pv[:, ds(j * 65, 65)],
                                pet[:, ds(j * SL, SL)],
                                start=(pc == 0),
                                stop=False,
                            )
                    pend = (c, et, vt[:, c, :])
                pc, pet, pv = pend
                for j, ut in enumerate((utA, utB)):
                    nc.tensor.matmul(
                        ut[:65, :],
                        pv[:, ds(j * 65, 65)],
                        pet[:, ds(j * SL, SL)],
                        start=False,
                        stop=True,
                    )
                norm_pend = (p, utA, utB)
            emit_normalize(*norm_pend)

            # ---- output projection + residual + LayerNorm ----
            out_sb = big.tile([P, SQ, D], FP32, tag="out")
            for i in range(SQ):
                for n in range(2):
                    pp = scp.tile([P, 1024], FP32, tag="sc")
                    for p in range(PAIRS):
                        nc.tensor.matmul(
                            pp[:, 0:512],
                            ctx_sb[:, p, ts(i, P)],
                            wo_sb[:, p, ds(n * 512, 512)],
                            start=(p == 0),
                            stop=(p == PAIRS - 1),
                        )
                    res = stream.tile([P, 512], FP32, tag="res")
                    nc.sync.dma_start(res[:], xqr_d[ts(i, P), ds(n * 512, 512)])
                    tmp = stream.tile([P, 512], FP32, tag="otmp")
                    nc.vector.tensor_tensor(tmp[:], pp[:, 0:512], res[:], ALU.add)
                    nc.vector.tensor_tensor(
                        out_sb[:, i, ds(n * 512, 512)],
                        tmp[:],
                        bo_b[:, ds(n * 512, 512)],
                        ALU.add,
                    )
                row = out_sb[:, i, :]
                stats = small.tile([P, 2, 6], FP32, tag="stats")
                nc.vector.bn_stats(stats[:, 0, :], row[:, 0:512])
                nc.vector.bn_stats(stats[:, 1, :], row[:, 512:1024])
                mv = small.tile([P, 2], FP32, tag="mv")
                nc.vector.bn_aggr(mv[:], stats[:])
                std = small.tile([P, 1], FP32, tag="std")
                nc.scalar.activation(
                    out=std[:], in_=mv[:, 1:2], func=AF.Sqrt, bias=eps_t[:], scale=1.0
                )
                rstd = small.tile([P, 1], FP32, tag="rstd")
                nc.vector.reciprocal(out=rstd[:], in_=std[:])
                ytile = wide.tile([P, D], FP32, tag="y")
                nc.vector.tensor_scalar(
                    out=ytile[:],
                    in0=row,
                    scalar1=mv[:, 0:1],
                    scalar2=rstd[:],
                    op0=ALU.subtract,
                    op1=ALU.mult,
                )
                nc.vector.tensor_tensor(ytile[:], ytile[:], gam_b[:], ALU.mult)
                nc.vector.tensor_tensor(ytile[:], ytile[:], bet_b[:], ALU.add)
                nc.sync.dma_start(y_d[ts(i, P), :], ytile[:])

    nc.compile()
    return nc


def get_nc():
    if "nc" not in _NC_CACHE:
        _NC_CACHE["nc"] = build_nc()
    return _NC_CACHE["nc"]


def kernel(
    query,
    key,
    value,
    Wq,
    bq,
    Wk,
    bk,
    Wv,
    bv,
    Wo,
    bo,
    ln_gamma,
    ln_beta,
    _trace=False,
    _trace_cores=None,
):
    import ml_dtypes

    BF = ml_dtypes.bfloat16
    query = np.ascontiguousarray(np.asarray(query, dtype=np.float32))
    key_bf = np.ascontiguousarray(np.asarray(key, np.float32)).astype(BF)
    value_bf = np.ascontiguousarray(np.asarray(value, np.float32)).astype(BF)
    query_bf = query.astype(BF)
    Wq = np.asarray(Wq, np.float32)
    Wk = np.asarray(Wk, np.float32)
    Wv = np.asarray(Wv, np.float32)
    Wo = np.asarray(Wo, np.float32)
    bq_r = np.ascontiguousarray(
        np.asarray(bq, np.float32).reshape(KC, P).T
    )  # [P, KC] with bias for dim 128k+p at [p, k]
    bk_f = np.asarray(bk, np.float32)
    bv_f = np.asarray(bv, np.float32)
    shared = {
        "wq": np.ascontiguousarray(Wq.astype(BF)),
        "wo": np.ascontiguousarray(Wo.astype(BF)),
        "bq": bq_r,
        "bo": np.ascontiguousarray(np.asarray(bo, np.float32)),
        "gam": np.ascontiguousarray(np.asarray(ln_gamma, np.float32)),
        "bet": np.ascontiguousarray(np.asarray(ln_beta, np.float32)),
        "ident": np.eye(P, dtype=BF),
        "ones": np.ones((P, 64), dtype=np.float32),
    }
    in_maps = []
    for c in range(N_CORES):
        b, g = divmod(c, NB)
        rows = slice(g * SL, (g + 1) * SL)
        cols = slice(g * OWN, (g + 1) * OWN)
        m = dict(shared)
        m["xq"] = np.ascontiguousarray(query_bf[b, rows, :])
        m["xqr"] = np.ascontiguousarray(query[b, rows, :])
        m["xk"] = key_bf[b]
        m["xv"] = value_bf[b]
        m["wk"] = np.ascontiguousarray(Wk[:, cols].astype(BF))
        m["wv"] = np.ascontiguousarray(Wv[:, cols].astype(BF))
        m["bk"] = np.ascontiguousarray(bk_f[cols].reshape(2, P).T)
        m["bv"] = np.ascontiguousarray(bv_f[cols])
        in_maps.append(m)

    nc = get_nc()
    res = run_bass_kernel_spmd(
        nc,
        in_maps,
        list(range(N_CORES)),
        trace=_trace,
        trace_cores=_trace_cores,
    )
    out = np.empty((B, S, D), dtype=np.float32)
    for c in range(N_CORES):
        b, g = divmod(c, NB)
        out[b, g * SL : (g + 1) * SL, :] = res.results[c]["y"]
    if _trace:
        return out, res
    return out
